# revision 59
# baseline (speedup 1.0000x reference)
# CATS-SwiGLU decode kernel for TRN2 (8 NeuronCores, SPMD tensor-parallel).
# v4: bf16 weights streamed to BOTH the TensorEngine and the Vector engine
# concurrently (each sustains ~120-130 Gelem/s; combined they outrun the
# ~430 GB/s per-core DMA fabric).  Three HWDGE queues: sync + gpsimd carry
# the gate/up stream (alternating pieces, deep prefetch pools so the DMA
# runs ahead of compute), and the scalar queue carries the down-projection
# stream, gated behind the gate matrix by a 2-byte SBUF->SBUF dummy DMA
# reading the gate's last DVE accumulator (the Tile list scheduler cannot
# hoist it, and the ACT engine stalling on the gate is harmless).
#
#   x1    = silu(x @ Wgatet); flags = |x1| > thr
#   z     = where(flags, (x @ Wup.T) * x1, 0);  out = z @ Wdownt
#
# f-split for gate/up: f-rows [0:FD) via DVE affine_mul_reduce over f-major
# tiles [128f, 4096d] (accumulator lands f-on-partitions, already
# transposed for the down GEMV); f-rows [FD:1376) via PE accumulating GEMV
# matmuls (stationary x column, weights moving).  d-split for down:
# d-cols [0:DP) on PE (f-major weights, z-column stationary), d-cols
# [DP:4096) on DVE (d-major weights times a ones-broadcast z).
import sys

for _p in ("/opt/trn_rl_repo",):
    if _p not in sys.path:
        sys.path.insert(0, _p)

import numpy as np
import ml_dtypes

import concourse.bass as bass
import concourse.tile as tile
from concourse import bacc, mybir
from concourse.bass_utils import run_bass_kernel_spmd
from concourse.masks import make_identity

D = 4096
FF = 11008
NCORES = 8
FSH = FF // NCORES            # 1376 rows of d_ff per core
NCD = D // 128                # 32 contraction chunks for gate/up PE part
NCF = (FSH + 127) // 128      # 11 f chunks
LASTF = FSH - 128 * (NCF - 1)  # 96 rows in the last f chunk

FDC = 6                       # f chunks on DVE for gate/up
FD = FDC * 128                # 768
FP = FSH - FD                 # 608 PE-part f width
FTP = ((0, 512), (512, FP - 512))   # PE-part psum f-tiles
NPP = 2                       # PE-part stream pieces per matrix (16 chunks)
CPP = NCD // NPP              # 16 chunks per PE piece
PPW = CPP * FP                # PE piece width (elems/partition)
TW = (2, 2, 1, 1)             # DVE stream tile widths in chunks: the last
TOFF = (0, 2, 4, 5)           # two are single-chunk so the end-of-stream
NPT = len(TW)                 # DVE chase is one affine, not two
PBUFS = 2                     # PE-piece prefetch depth
TBUFS = 2                     # DVE-tile prefetch depth

DP = 2304                     # d-cols on PE for down
DTILES = ((0, 512), (512, 512), (1024, 512), (1536, 512), (2048, 256))
NDVG = (D - DP) // 128        # 14 DVE down groups
WDPB = (3, 3, 3, 2)           # wdp stream pieces, in f-chunks
WDDB = (4, 4, 4, 2)           # wdd stream pieces, in d-groups (small tail)

F32 = mybir.dt.float32
BF16 = mybir.dt.bfloat16
BF = ml_dtypes.bfloat16

_CACHE = {}


def _bcast(ap, parts):
    return bass.AP(tensor=ap.tensor, offset=ap.offset, ap=[[0, parts]] + list(ap.ap))


def _build_nc():
    nc = bacc.Bacc("TRN2", target_bir_lowering=False, debug=False)

    xf_d = nc.dram_tensor("xf", [128, D + NCD], BF16, kind="ExternalInput")
    wgd_d = nc.dram_tensor("wgd", [128, FDC * D], BF16, kind="ExternalInput")
    wud_d = nc.dram_tensor("wud", [128, FDC * D], BF16, kind="ExternalInput")
    wgp_d = nc.dram_tensor("wgp", [128, NCD * FP], BF16, kind="ExternalInput")
    wup_d = nc.dram_tensor("wup", [128, NCD * FP], BF16, kind="ExternalInput")
    wdp_d = nc.dram_tensor("wdp", [128, NCF * DP], BF16, kind="ExternalInput")
    wdd_d = nc.dram_tensor("wdd", [128, NDVG * FSH], BF16, kind="ExternalInput")
    thr_d = nc.dram_tensor("thr", [1], F32, kind="ExternalInput")
    outp_d = nc.dram_tensor("outp", [DP], F32, kind="ExternalOutput")
    outd_d = nc.dram_tensor("outd", [128, NDVG], F32, kind="ExternalOutput")

    with tile.TileContext(nc) as tc:
        with (
            tc.tile_pool(name="const", bufs=1) as cp,
            tc.tile_pool(name="ppool", bufs=PBUFS) as ppool,
            tc.tile_pool(name="tpool", bufs=TBUFS) as tpool,
            tc.tile_pool(name="acts", bufs=1) as acts,
        ):
            # x (replicated + column-chunked, one wide-row descriptor) is
            # the FIRST transfer on the sync ring: every DVE affine and PE
            # matmul needs it, and tiny-row descriptors starve when the
            # weight queues are busy.  thr rides the idle scalar ring.
            xboth = cp.tile([128, D + NCD], BF16)
            nc.sync.dma_start(out=xboth[:], in_=xf_d.ap())
            xrep = xboth[:, 0:D]
            xcol = xboth[:, D : D + NCD]
            thr_sb = cp.tile([128, 1], F32)
            nc.scalar.dma_start(out=thr_sb[:], in_=_bcast(thr_d.ap(), 128))
            ones_col = cp.tile([1, 128], BF16)
            nc.vector.memset(ones_col[:], 1.0)
            ident = cp.tile([128, 128], BF16)
            make_identity(nc, ident[:])

            # activation scratch
            accg = acts.tile([128, FDC], F32)   # DVE-part gate accum
            accu = acts.tile([128, FDC], F32)   # DVE-part up accum
            sgd = acts.tile([128, FDC], F32)
            x1d = acts.tile([128, FDC], F32)
            abd = acts.tile([128, FDC], F32)
            mkd = acts.tile([128, FDC], F32)
            xmd = acts.tile([128, FDC], F32)
            sgp = acts.tile([1, FP], F32)
            x1p = acts.tile([1, FP], F32)
            abp = acts.tile([1, FP], F32)
            mkp = acts.tile([1, FP], F32)
            xmp = acts.tile([1, FP], F32)
            z_row = acts.tile([1, NCF * 128], BF16)
            nc.vector.memset(z_row[:], 0.0)
            z_bf = acts.tile([128, NCF], BF16)
            nc.vector.memset(z_bf[:], 0.0)
            zrep = acts.tile([128, FSH], BF16)
            osbp = acts.tile([1, DP], F32)
            outd_sb = acts.tile([128, NDVG], F32)
            warm = acts.tile([128, 1], F32)
            nc.scalar.activation(
                warm[:], thr_sb[:], mybir.ActivationFunctionType.Sigmoid
            )
            nc.scalar.activation(
                warm[:], thr_sb[:], mybir.ActivationFunctionType.Abs
            )
            nc.scalar.copy(warm[:], thr_sb[:])

            # resident down-weight tiles; their DMAs ride the scalar queue,
            # pinned behind the gate/up stream by strided dummy DMAs
            wdp_sb = acts.tile([128, NCF * DP], BF16)
            wdd_sb = acts.tile([128, NDVG * FSH], BF16)

            # ---- gate/up weight stream ----
            # per-matrix piece order and queue map, byte-balanced so the
            # gate matrix fully lands before the up matrix needs bandwidth
            ORDER = (
                ("T", 0), ("P", 0), ("T", 1), ("P", 1), ("T", 2), ("T", 3)
            )
            qs = (nc.sync, nc.gpsimd)
            ptiles = {}
            dtiles = {}

            def emit_piece(mi, kind, idx, oi):
                q = qs[0]
                if kind == "P":
                    dram = wgp_d if mi == 0 else wup_d
                    t = ppool.tile([128, PPW], BF16, tag="pw", name="pw")
                    q.dma_start(
                        out=t[:], in_=dram.ap()[:, idx * PPW : (idx + 1) * PPW]
                    )
                    ptiles[(mi, idx)] = t
                else:
                    dram = wgd_d if mi == 0 else wud_d
                    t = tpool.tile([128, 2 * D], BF16, tag="tw", name="tw")
                    c0, w = TOFF[idx], TW[idx]
                    q.dma_start(
                        out=t[:, 0 : w * D],
                        in_=dram.ap()[:, c0 * D : (c0 + w) * D],
                    )
                    dtiles[(mi, idx)] = t

            stream = []
            for mi in range(2):
                for oi, (kind, idx) in enumerate(ORDER):
                    stream.append((mi, kind, idx, oi))

            with tc.tile_pool(name="ps1", bufs=1, space="PSUM") as ps1:
                pgp = ps1.tile([1, 1024], F32)
                pup = ps1.tile([1, 1024], F32)
                pzrow = ps1.tile([1, FDC * 128], F32)
                pzcol = ps1.tile([128, NCF - FDC], F32)

                # greedy prefetch bounded by per-kind pool depth
                emitted = 0
                inflight = {"P": 0, "T": 0}
                bufs = {"P": PBUFS, "T": TBUFS}

                def pump():
                    nonlocal emitted
                    while emitted < len(stream):
                        mi, kind, idx, oi = stream[emitted]
                        if inflight[kind] >= bufs[kind]:
                            return
                        emit_piece(mi, kind, idx, oi)
                        inflight[kind] += 1
                        emitted += 1

                def consume(mi, kind, idx, oi):
                    if kind == "P":
                        accp = pgp if mi == 0 else pup
                        t = ptiles[(mi, idx)]
                        for cc in range(CPP):
                            c = idx * CPP + cc
                            for toff, tlen in FTP:
                                nc.tensor.matmul(
                                    out=accp[0:1, toff : toff + tlen],
                                    lhsT=xcol[:, c : c + 1],
                                    rhs=t[:, cc * FP + toff : cc * FP + toff + tlen],
                                    start=(c == 0),
                                    stop=(c == NCD - 1),
                                )
                    else:
                        acct = accg if mi == 0 else accu
                        t = dtiles[(mi, idx)]
                        for j in range(TW[idx]):
                            c = TOFF[idx] + j
                            nc.vector.affine_mul_reduce(
                                out=t[:, j * D : (j + 1) * D],
                                accum_out=acct[:, c : c + 1],
                                in0=t[:, j * D : (j + 1) * D],
                                in1=xrep[:],
                                scale=1.0,
                                bias=0.0,
                            )

                def emit_wd_dmas():
                    # dummy DMAs touching the first element of each wd
                    # stream piece, reading the up matrix's second-to-last
                    # DVE tile: the WAW dependency pins every wd piece DMA
                    # behind the bulk of the gate/up stream, so the list
                    # scheduler cannot hoist them into it.
                    trig = ptiles[(1, NPP - 1)]
                    nc.scalar.dma_start(
                        out=wdp_sb[0:1, 0 : 9 * DP + 1 : 3 * DP],
                        in_=trig[0:1, 0:4],
                    )
                    nc.scalar.dma_start(
                        out=wdd_sb[0:1, 0 : 12 * FSH + 1 : 4 * FSH],
                        in_=trig[0:1, 4:8],
                    )
                    po = do = 0
                    for k in range(4):
                        pw = WDPB[k] * DP
                        nc.scalar.dma_start(
                            out=wdp_sb[:, po * DP : po * DP + pw],
                            in_=wdp_d.ap()[:, po * DP : po * DP + pw],
                        )
                        po += WDPB[k]
                        dw = WDDB[k] * FSH
                        nc.scalar.dma_start(
                            out=wdd_sb[:, do * FSH : do * FSH + dw],
                            in_=wdd_d.ap()[:, do * FSH : do * FSH + dw],
                        )
                        do += WDDB[k]

                pump()
                for k, item in enumerate(stream):
                    consume(*item)
                    inflight[item[1]] -= 1
                    pump()
                    if k == len(stream) - 1:
                        emit_wd_dmas()

                # ---- gate elementwise ----
                nc.scalar.activation(
                    sgd[:], accg[:], mybir.ActivationFunctionType.Sigmoid
                )
                nc.vector.tensor_mul(x1d[:], accg[:], sgd[:])
                nc.scalar.activation(
                    abd[:], x1d[:], mybir.ActivationFunctionType.Abs
                )
                nc.vector.tensor_scalar(
                    out=mkd[:], in0=abd[:], scalar1=thr_sb[:], scalar2=None,
                    op0=mybir.AluOpType.is_gt,
                )
                nc.vector.tensor_mul(xmd[:], x1d[:], mkd[:])
                nc.scalar.activation(
                    sgp[:], pgp[0:1, 0:FP], mybir.ActivationFunctionType.Sigmoid
                )
                nc.vector.tensor_mul(x1p[:], pgp[0:1, 0:FP], sgp[:])
                nc.scalar.activation(
                    abp[:], x1p[:], mybir.ActivationFunctionType.Abs
                )
                nc.vector.tensor_scalar(
                    out=mkp[:], in0=abp[:], scalar1=thr_sb[0:1, :], scalar2=None,
                    op0=mybir.AluOpType.is_gt,
                )
                nc.vector.tensor_mul(xmp[:], x1p[:], mkp[:])

                # ---- z ----
                nc.vector.tensor_mul(z_bf[:, 0:FDC], accu[:], xmd[:])
                nc.vector.tensor_mul(
                    z_row[0:1, FD:FSH], pup[0:1, 0:FP], xmp[:]
                )
                # DVE-part z to row form (for the z broadcast)
                for c in range(FDC):
                    nc.tensor.matmul(
                        out=pzrow[0:1, c * 128 : (c + 1) * 128],
                        lhsT=z_bf[:, c : c + 1],
                        rhs=ident[:],
                        start=True,
                        stop=True,
                    )
                nc.scalar.copy(z_row[0:1, 0:FD], pzrow[0:1, 0:FD])
                # PE-part z to column form
                for c in range(FDC, NCF):
                    pc = 128 if c < NCF - 1 else LASTF
                    nc.tensor.matmul(
                        out=pzcol[0:pc, c - FDC : c - FDC + 1],
                        lhsT=z_row[0:1, c * 128 : c * 128 + pc],
                        rhs=ones_col[0:1, 0:1],
                        start=True,
                        stop=True,
                    )
                nc.scalar.copy(z_bf[:, FDC:NCF], pzcol[:, 0 : NCF - FDC])

            with tc.tile_pool(name="ps2", bufs=1, space="PSUM") as ps2:
                pdp = ps2.tile([1, DP], F32)
                przep = ps2.tile([128, 1536], F32)
                for toff, tlen in ((0, 512), (512, 512), (1024, FSH - 1024)):
                    nc.tensor.matmul(
                        out=przep[:, toff : toff + tlen],
                        lhsT=ones_col[:],
                        rhs=z_row[0:1, toff : toff + tlen],
                        start=True,
                        stop=True,
                    )
                nc.scalar.copy(zrep[:], przep[:, 0:FSH])

                # PE and DVE down parts, chasing the wd stream pieces
                for c in range(NCF):
                    pc = 128 if c < NCF - 1 else LASTF
                    for toff, tlen in DTILES:
                        nc.tensor.matmul(
                            out=pdp[0:1, toff : toff + tlen],
                            lhsT=z_bf[0:pc, c : c + 1],
                            rhs=wdp_sb[0:pc, c * DP + toff : c * DP + toff + tlen],
                            start=(c == 0),
                            stop=(c == NCF - 1),
                        )
                for g in range(NDVG):
                    nc.vector.affine_mul_reduce(
                        out=wdd_sb[:, g * FSH : (g + 1) * FSH],
                        accum_out=outd_sb[:, g : g + 1],
                        in0=wdd_sb[:, g * FSH : (g + 1) * FSH],
                        in1=zrep[:],
                        scale=1.0,
                        bias=0.0,
                    )
                for toff, tlen in DTILES:
                    sl = slice(toff, toff + tlen)
                    nc.scalar.copy(osbp[0:1, sl], pdp[0:1, sl])

            nc.sync.dma_start(out=outp_d.ap(), in_=osbp[:])
            nc.sync.dma_start(
                out=outd_d.ap()[:, 0 : NDVG // 2], in_=outd_sb[:, 0 : NDVG // 2]
            )
            nc.sync.dma_start(
                out=outd_d.ap()[:, NDVG // 2 : NDVG],
                in_=outd_sb[:, NDVG // 2 : NDVG],
            )

    nc.compile()
    return nc


def _get_nc():
    if "nc" not in _CACHE:
        _CACHE["nc"] = _build_nc()
    return _CACHE["nc"]


def make_in_maps(x, Wup, Wgatet, Wdownt, threshold):
    """Shard full inputs into the 8 per-core input maps (bf16 weights)."""
    x_flat = np.asarray(x, dtype=np.float32).reshape(D)
    xcol = np.ascontiguousarray(x_flat.reshape(NCD, 128).T).astype(BF)
    xf = np.ascontiguousarray(
        np.concatenate(
            [np.broadcast_to(x_flat.astype(BF), (128, D)), xcol], axis=1
        )
    )
    thr = np.asarray(threshold, dtype=np.float32).reshape(1)
    Wup = np.asarray(Wup, dtype=np.float32)
    Wgatet = np.asarray(Wgatet, dtype=np.float32)
    Wdownt = np.asarray(Wdownt, dtype=np.float32)
    in_maps = []
    for i in range(NCORES):
        sl = slice(i * FSH, (i + 1) * FSH)
        wg_slice = Wgatet[:, sl]                  # [D, FSH] d-major
        wu_slice = Wup[sl, :]                     # [FSH, D] f-major
        wd_slice = Wdownt[sl, :]                  # [FSH, D] f-major

        wgT = np.ascontiguousarray(wg_slice.T)    # [FSH, D] f-major
        wgd = (
            wgT[:FD].reshape(FDC, 128, D).transpose(1, 0, 2).reshape(128, FDC * D)
        )
        wud = (
            wu_slice[:FD]
            .reshape(FDC, 128, D)
            .transpose(1, 0, 2)
            .reshape(128, FDC * D)
        )
        wgp = (
            wg_slice[:, FD:]
            .reshape(NCD, 128, FP)
            .transpose(1, 0, 2)
            .reshape(128, NCD * FP)
        )
        wuT = np.ascontiguousarray(wu_slice.T)    # [D, FSH] d-major
        wup = (
            wuT[:, FD:]
            .reshape(NCD, 128, FP)
            .transpose(1, 0, 2)
            .reshape(128, NCD * FP)
        )
        wd_pad = np.zeros((NCF * 128, DP), dtype=np.float32)
        wd_pad[:FSH] = wd_slice[:, :DP]
        wdp = (
            wd_pad.reshape(NCF, 128, DP).transpose(1, 0, 2).reshape(128, NCF * DP)
        )
        wdT = np.ascontiguousarray(wd_slice.T)    # [D, FSH] d-major
        wdd = (
            wdT[DP:]
            .reshape(NDVG, 128, FSH)
            .transpose(1, 0, 2)
            .reshape(128, NDVG * FSH)
        )
        in_maps.append(
            {
                "xf": xf,
                "wgd": np.ascontiguousarray(wgd).astype(BF),
                "wud": np.ascontiguousarray(wud).astype(BF),
                "wgp": np.ascontiguousarray(wgp).astype(BF),
                "wup": np.ascontiguousarray(wup).astype(BF),
                "wdp": np.ascontiguousarray(wdp).astype(BF),
                "wdd": np.ascontiguousarray(wdd).astype(BF),
                "thr": thr,
            }
        )
    return in_maps


def run_sharded(x, Wup, Wgatet, Wdownt, threshold, trace=False, tmpdir=None):
    """Run on the 8 NeuronCores; returns (full_output, BassKernelResults)."""
    nc = _get_nc()
    in_maps = make_in_maps(x, Wup, Wgatet, Wdownt, threshold)
    res = run_bass_kernel_spmd(
        nc, in_maps, list(range(NCORES)), trace=trace, tmpdir=tmpdir
    )
    acc = np.zeros(D, dtype=np.float64)
    for r in res.results:
        acc[:DP] += r["outp"].reshape(DP).astype(np.float64)
        acc[DP:] += r["outd"].T.reshape(D - DP).astype(np.float64)
    out = acc.astype(np.float32).reshape(1, 1, D)
    return out, res


def kernel(x, Wup, Wgatet, Wdownt, threshold):
    out, _ = run_sharded(x, Wup, Wgatet, Wdownt, threshold)
    return out
